# revision 1
# baseline (speedup 1.0000x reference)
"""Additive attention (Bahdanau-style) TRN2 Bass kernel, SPMD over 8 NeuronCores.

Reference computation (B=4, Lq=Lk=512, D=H=128):
    q = queries @ Wq                     (B, Lq, H)
    k = keys @ Wk                        (B, Lk, H)
    scores[b,i,j] = sum_h wv[h] * tanh(q[b,i,h] + k[b,j,h])
    scores masked to -1e6 for j >= valid_seq_len[b] -> softmax over j -> @ values @ Wo

Sharding: data-parallel over Lq (each core takes 64 queries of EVERY batch,
so the per-core work is Sum_b valid_b * 64 regardless of the mask skew).
The kernel is specialized at build time to the actual valid_seq_len values
(masked key columns are simply never computed; exp() of a masked column is
exactly 0 in the reference, so skipping them is exact).

Per-core device program (h lives on SBUF partitions):
  qfT (h,i) = Wq^T @ qT,  kfT_b (h,j) = Wk^T @ kT_b          [PE]
  S chunk (h, G, V) = kfT broadcast + qfT broadcast          [DVE, stride-0 APs]
  F = tanh(S)                                                [ACT]
  scores rows: M=32 matmuls with a shifted-diagonal wv matrix Z so query r
    lands on PSUM partition r (accumulating +0 rows elsewhere)  [PE]
  softmax: reduce_max(negate) -> Exp(bias=-max, accum_out=rowsum)  [DVE+ACT]
  attn^T via PE transpose; PV accumulated over j-tiles; out = (pvT)^T@Wo,
  with the 1/rowsum folded into the final PSUM->SBUF copy as a per-row scale.
"""

import math
from contextlib import ExitStack

import numpy as np

B, LQ, LK, D, H = 4, 512, 512, 128, 128
NCORES = 8
QPC = LQ // NCORES  # queries per core per batch = 64
G = 8  # queries per DVE/ACT chunk

_RUNNERS: dict = {}


def _build_program(valid: tuple):
    import concourse.bacc as bacc
    import concourse.mybir as mybir
    import concourse.tile as tile

    f32 = mybir.dt.float32
    AF = mybir.ActivationFunctionType

    nc = bacc.Bacc("TRN2", target_bir_lowering=False, debug=False)

    qT_d = nc.dram_tensor("qT", [D, B * QPC], f32, kind="ExternalInput")
    kT_d = nc.dram_tensor("kT", [B * D, LK], f32, kind="ExternalInput")
    vals_d = nc.dram_tensor("vals", [B * LK, D], f32, kind="ExternalInput")
    wq_d = nc.dram_tensor("wq", [D, H], f32, kind="ExternalInput")
    wk_d = nc.dram_tensor("wk", [D, H], f32, kind="ExternalInput")
    wo_d = nc.dram_tensor("wo", [D, H], f32, kind="ExternalInput")
    zmat_d = nc.dram_tensor("zmat", [H, 63], f32, kind="ExternalInput")
    ident_d = nc.dram_tensor("ident", [128, 128], f32, kind="ExternalInput")
    out_d = nc.dram_tensor("out", [B * QPC, H], f32, kind="ExternalOutput")

    njs = [max(1, math.ceil(v / 128)) for v in valid]

    with tile.TileContext(nc) as tc, ExitStack() as ctx:
        consts = ctx.enter_context(tc.tile_pool(name="consts", bufs=1))

        wq_sb = consts.tile([D, H], f32, tag="wq")
        nc.sync.dma_start(wq_sb[:], wq_d[:])
        wk_sb = consts.tile([D, H], f32, tag="wk")
        nc.sync.dma_start(wk_sb[:], wk_d[:])
        wo_sb = consts.tile([D, H], f32, tag="wo")
        nc.sync.dma_start(wo_sb[:], wo_d[:])
        zmat_sb = consts.tile([H, 63], f32, tag="zmat")
        nc.sync.dma_start(zmat_sb[:], zmat_d[:])
        ident_sb = consts.tile([128, 128], f32, tag="ident")
        nc.sync.dma_start(ident_sb[:], ident_d[:])
        qT_sb = consts.tile([D, B * QPC], f32, tag="qT")
        nc.sync.dma_start(qT_sb[:], qT_d[:])

        kT_sb = []
        for b in range(B):
            t = consts.tile([D, LK], f32, tag=f"kT{b}")
            nc.sync.dma_start(t[:], kT_d[b * D : (b + 1) * D, :])
            kT_sb.append(t)

        vals_sb = {}
        for b in range(B):
            for jt in range(njs[b]):
                t = consts.tile([128, D], f32, tag=f"vals{b}_{jt}")
                r0 = b * LK + jt * 128
                nc.sync.dma_start(t[:], vals_d[r0 : r0 + 128, :])
                vals_sb[(b, jt)] = t

        # ---- projections: qfT (h, B*QPC), kfT_b (h, LK) ----
        qfT_sb = consts.tile([H, B * QPC], f32, tag="qfT")
        kfT_sb = []
        with tc.tile_pool(name="proj_ps", bufs=2, space="PSUM") as proj_ps:
            qf_ps = proj_ps.tile([H, B * QPC], f32, tag="qf")
            nc.tensor.matmul(qf_ps[:], lhsT=wq_sb[:], rhs=qT_sb[:], start=True, stop=True)
            nc.scalar.copy(qfT_sb[:], qf_ps[:])
            for b in range(B):
                kf_ps = proj_ps.tile([H, LK], f32, tag="kf")
                nc.tensor.matmul(kf_ps[:], lhsT=wk_sb[:], rhs=kT_sb[b][:], start=True, stop=True)
                t = consts.tile([H, LK], f32, tag=f"kfT{b}")
                nc.scalar.copy(t[:], kf_ps[:])
                kfT_sb.append(t)

        spool = ctx.enter_context(tc.tile_pool(name="s", bufs=3))
        fpool = ctx.enter_context(tc.tile_pool(name="f", bufs=3))
        scpool = ctx.enter_context(tc.tile_pool(name="scores", bufs=2, space="PSUM"))
        epool = ctx.enter_context(tc.tile_pool(name="e", bufs=2))
        stat = ctx.enter_context(tc.tile_pool(name="stat", bufs=8))
        tpool = ctx.enter_context(tc.tile_pool(name="attnT", bufs=4))
        tps = ctx.enter_context(tc.tile_pool(name="tps", bufs=2, space="PSUM"))
        pvps = ctx.enter_context(tc.tile_pool(name="pvps", bufs=1, space="PSUM"))
        outps = ctx.enter_context(tc.tile_pool(name="outps", bufs=2, space="PSUM"))
        opool = ctx.enter_context(tc.tile_pool(name="osb", bufs=2))

        pv_ps = pvps.tile([D, B * QPC], f32, tag="pv")
        rinvs = []

        for b in range(B):
            V = valid[b]
            nj = njs[b]
            sc_ps = scpool.tile([QPC, 512], f32, tag="sc")

            for g in range(QPC // G):
                S = spool.tile([H, G, V], f32, tag="s")
                kb = kT_broadcast = kfT_sb[b][:, 0:V].unsqueeze(1).broadcast_to([H, G, V])
                qc = (
                    qfT_sb[:, b * QPC + g * G : b * QPC + (g + 1) * G]
                    .unsqueeze(2)
                    .broadcast_to([H, G, V])
                )
                nc.vector.tensor_add(S[:], kb, qc)
                F = fpool.tile([H, G, V], f32, tag="f")
                nc.scalar.activation(F[:], S[:], AF.Tanh)
                for r in range(G):
                    qi = g * G + r
                    grp, row = divmod(qi, 32)
                    nc.tensor.matmul(
                        sc_ps[32 * grp : 32 * grp + 32, 0:V],
                        lhsT=zmat_sb[:, 31 - row : 63 - row],
                        rhs=F[:, r, :],
                        start=(row == 0),
                        stop=(row == 31),
                    )

            # ---- softmax over j (free axis), unnormalized ----
            negmax = stat.tile([QPC, 1], f32, tag="negmax")
            nc.vector.reduce_max(
                negmax[:], sc_ps[:, 0:V], axis=mybir.AxisListType.X, negate=True
            )
            E = epool.tile([QPC, 512], f32, tag="e")
            if V < nj * 128:
                nc.vector.memset(E[:, V : nj * 128], 0.0)
            rowsum = stat.tile([QPC, 1], f32, tag="rowsum")
            nc.scalar.activation(
                E[:, 0:V], sc_ps[:, 0:V], AF.Exp, bias=negmax[:], accum_out=rowsum[:]
            )
            rinv = stat.tile([QPC, 1], f32, tag=f"rinv{b}")
            nc.vector.reciprocal(rinv[:], rowsum[:])
            rinvs.append(rinv)

            # ---- attn^T tiles and PV accumulation ----
            for jt in range(nj):
                at_ps = tps.tile([128, QPC], f32, tag="atps")
                nc.tensor.transpose(
                    at_ps[:], E[:, 128 * jt : 128 * (jt + 1)], ident_sb[0:QPC, 0:QPC]
                )
                at_sb = tpool.tile([128, QPC], f32, tag="atsb")
                nc.scalar.copy(at_sb[:], at_ps[:])
                nc.tensor.matmul(
                    pv_ps[:, b * QPC : (b + 1) * QPC],
                    lhsT=vals_sb[(b, jt)][:],
                    rhs=at_sb[:],
                    start=(jt == 0),
                    stop=(jt == nj - 1),
                )

        # ---- output projection, with 1/rowsum folded into the PSUM->SBUF copy ----
        pv_sb = consts.tile([D, B * QPC], f32, tag="pvsb")
        nc.scalar.copy(pv_sb[:], pv_ps[:])
        for b in range(B):
            o_ps = outps.tile([QPC, H], f32, tag="ops")
            nc.tensor.matmul(
                o_ps[:],
                lhsT=pv_sb[:, b * QPC : (b + 1) * QPC],
                rhs=wo_sb[:],
                start=True,
                stop=True,
            )
            o_sb = opool.tile([QPC, H], f32, tag="osb")
            nc.scalar.activation(o_sb[:], o_ps[:], AF.Copy, scale=rinvs[b][:])
            nc.sync.dma_start(out_d[b * QPC : (b + 1) * QPC, :], o_sb[:])

    nc.compile()
    return nc


def _get_runner(valid: tuple):
    if valid not in _RUNNERS:
        _RUNNERS[valid] = _build_program(valid)
    return _RUNNERS[valid]


def make_in_maps(queries, keys, values, valid_seq_len, Wq, Wk, wv, Wo):
    queries = np.asarray(queries, np.float32)
    keys = np.asarray(keys, np.float32)
    values = np.asarray(values, np.float32)
    Wq = np.ascontiguousarray(np.asarray(Wq, np.float32))
    Wk = np.ascontiguousarray(np.asarray(Wk, np.float32))
    wv = np.asarray(wv, np.float32)
    Wo = np.ascontiguousarray(np.asarray(Wo, np.float32))

    qT_full = np.ascontiguousarray(queries.transpose(2, 0, 1))  # (D, B, Lq)
    kT = np.ascontiguousarray(keys.transpose(0, 2, 1)).reshape(B * D, LK)
    vals = np.ascontiguousarray(values.reshape(B * LK, D))
    zmat = np.zeros((H, 63), np.float32)
    zmat[:, 31] = wv
    ident = np.eye(128, dtype=np.float32)

    in_maps = []
    for c in range(NCORES):
        qT_c = np.ascontiguousarray(
            qT_full[:, :, c * QPC : (c + 1) * QPC].reshape(D, B * QPC)
        )
        in_maps.append(
            dict(qT=qT_c, kT=kT, vals=vals, wq=Wq, wk=Wk, wo=Wo, zmat=zmat, ident=ident)
        )
    return in_maps


def assemble(outs):
    out = np.empty((B, LQ, H), np.float32)
    for c in range(NCORES):
        out[:, c * QPC : (c + 1) * QPC, :] = outs[c].reshape(B, QPC, H)
    return out


def kernel(queries, keys, values, valid_seq_len, Wq, Wk, wv, Wo):
    from concourse import bass_utils

    valid = tuple(int(v) for v in np.asarray(valid_seq_len))
    nc = _get_runner(valid)
    in_maps = make_in_maps(queries, keys, values, valid_seq_len, Wq, Wk, wv, Wo)
    res = bass_utils.run_bass_kernel_spmd(nc, in_maps, core_ids=list(range(NCORES)))
    return assemble([res.results[c]["out"] for c in range(NCORES)])


# revision 3
# speedup vs baseline: 737.6688x; 737.6688x over previous
"""Additive attention (Bahdanau-style) TRN2 Bass kernel, SPMD over 8 NeuronCores.

Reference computation (B=4, Lq=Lk=512, D=H=128):
    q = queries @ Wq                     (B, Lq, H)
    k = keys @ Wk                        (B, Lk, H)
    scores[b,i,j] = sum_h wv[h] * tanh(q[b,i,h] + k[b,j,h])
    scores masked to -1e6 for j >= valid_seq_len[b] -> softmax over j -> @ values @ Wo

Sharding: data-parallel over Lq (each core takes 64 queries of EVERY batch,
so the per-core work is Sum_b valid_b * 64 regardless of the mask skew).
The kernel is specialized at build time to the actual valid_seq_len values
(masked key columns are simply never computed; exp() of a masked column is
exactly 0 in the reference because exp(-1e6 - max) underflows, so skipping
them is exact).

Per-core device program (h lives on SBUF partitions):
  qfT (h,i) = Wq^T @ qT,  kfT_b (h,j) = Wk^T @ kT_b          [PE]
  S chunk (h, G, V) = kfT broadcast + qfT broadcast          [DVE, stride-0 APs]
  F = tanh(S)                                                [ACT]
  scores rows: M=32 matmuls with a shifted-diagonal wv matrix Z so query r
    lands on PSUM partition r (accumulating +0 rows elsewhere)  [PE]
  softmax: reduce_max(negate) -> Exp(bias=-max, accum_out=rowsum)  [DVE+ACT]
  attn^T via PE transpose; PV accumulated over j-tiles; out = (pvT)^T@Wo,
  with the 1/rowsum folded into the final PSUM->SBUF copy as a per-row scale.
"""

import math
from contextlib import ExitStack

import numpy as np

B, LQ, LK, D, H = 4, 512, 512, 128, 128
NCORES = 8
QPC = LQ // NCORES  # queries per core per batch = 64
G = 8  # queries per DVE/ACT chunk

_RUNNERS: dict = {}


def _emit_body(nc, tc, ctx, consts, valid, njs, dram, f32, AF, AX):
    """One full attention pass. Safe to emit inside a For_i (idempotent)."""
    qT_d, kT_d, vals_d, wq_d, wk_d, wo_d, zmat_d, ident_d, out_d = dram

    wq_sb = consts.tile([D, H], f32, tag="wq")
    nc.sync.dma_start(wq_sb[:], wq_d[:])
    wk_sb = consts.tile([D, H], f32, tag="wk")
    nc.sync.dma_start(wk_sb[:], wk_d[:])
    wo_sb = consts.tile([D, H], f32, tag="wo")
    nc.sync.dma_start(wo_sb[:], wo_d[:])
    zmat_sb = consts.tile([H, 63], f32, tag="zmat")
    nc.sync.dma_start(zmat_sb[:], zmat_d[:])
    ident_sb = consts.tile([128, 128], f32, tag="ident")
    nc.sync.dma_start(ident_sb[:], ident_d[:])
    qT_sb = consts.tile([D, B * QPC], f32, tag="qT")
    nc.sync.dma_start(qT_sb[:], qT_d[:])

    kT_sb = []
    for b in range(B):
        t = consts.tile([D, LK], f32, tag=f"kT{b}")
        nc.sync.dma_start(t[:], kT_d[b * D : (b + 1) * D, :])
        kT_sb.append(t)

    vals_sb = {}
    for b in range(B):
        for jt in range(njs[b]):
            t = consts.tile([128, D], f32, tag=f"vals{b}_{jt}")
            r0 = b * LK + jt * 128
            nc.sync.dma_start(t[:], vals_d[r0 : r0 + 128, :])
            vals_sb[(b, jt)] = t

    # ---- projections: qfT (h, B*QPC), kfT_b (h, LK) ----
    qfT_sb = consts.tile([H, B * QPC], f32, tag="qfT")
    kfT_sb = []
    with tc.tile_pool(name="proj_ps", bufs=1, space="PSUM") as proj_ps:
        qf_ps = proj_ps.tile([H, B * QPC], f32, tag="qf")
        nc.tensor.matmul(qf_ps[:], lhsT=wq_sb[:], rhs=qT_sb[:], start=True, stop=True)
        nc.scalar.copy(qfT_sb[:], qf_ps[:])
        for b in range(B):
            kf_ps = proj_ps.tile([H, LK], f32, tag=f"kf{b}")
            nc.tensor.matmul(
                kf_ps[:], lhsT=wk_sb[:], rhs=kT_sb[b][:], start=True, stop=True
            )
            t = consts.tile([H, LK], f32, tag=f"kfT{b}")
            nc.scalar.copy(t[:], kf_ps[:])
            kfT_sb.append(t)

    spool = ctx.enter_context(tc.tile_pool(name="s", bufs=3))
    fpool = ctx.enter_context(tc.tile_pool(name="f", bufs=3))
    scpool = ctx.enter_context(tc.tile_pool(name="scores", bufs=2, space="PSUM"))
    epool = ctx.enter_context(tc.tile_pool(name="e", bufs=2))
    stat = ctx.enter_context(tc.tile_pool(name="stat", bufs=8))
    tpool = ctx.enter_context(tc.tile_pool(name="attnT", bufs=4))
    tps = ctx.enter_context(tc.tile_pool(name="tps", bufs=2, space="PSUM"))
    pvps = ctx.enter_context(tc.tile_pool(name="pvps", bufs=1, space="PSUM"))
    opool = ctx.enter_context(tc.tile_pool(name="osb", bufs=2))

    pv_ps = pvps.tile([D, B * QPC], f32, tag="pv")
    rinvs = []

    for b in range(B):
        V = valid[b]
        nj = njs[b]
        sc_ps = scpool.tile([QPC, 512], f32, tag="sc")

        for g in range(QPC // G):
            S = spool.tile([H, G, V], f32, tag="s")
            kb = kfT_sb[b][:, 0:V].unsqueeze(1).broadcast_to([H, G, V])
            qc = (
                qfT_sb[:, b * QPC + g * G : b * QPC + (g + 1) * G]
                .unsqueeze(2)
                .broadcast_to([H, G, V])
            )
            nc.vector.tensor_add(S[:], kb, qc)
            F = fpool.tile([H, G, V], f32, tag="f")
            nc.scalar.activation(F[:], S[:], AF.Tanh)
            for r in range(G):
                qi = g * G + r
                grp, row = divmod(qi, 32)
                nc.tensor.matmul(
                    sc_ps[32 * grp : 32 * grp + 32, 0:V],
                    lhsT=zmat_sb[:, 31 - row : 63 - row],
                    rhs=F[:, r, :],
                    start=(row == 0),
                    stop=(row == 31),
                )

        # ---- softmax over j (free axis), unnormalized ----
        negmax = stat.tile([QPC, 1], f32, tag="negmax")
        nc.vector.reduce_max(negmax[:], sc_ps[:, 0:V], axis=AX.X, negate=True)
        E = epool.tile([QPC, 512], f32, tag="e")
        if V < nj * 128:
            nc.vector.memset(E[:, V : nj * 128], 0.0)
        rowsum = stat.tile([QPC, 1], f32, tag="rowsum")
        nc.scalar.activation(
            E[:, 0:V], sc_ps[:, 0:V], AF.Exp, bias=negmax[:], accum_out=rowsum[:]
        )
        rinv = stat.tile([QPC, 1], f32, tag=f"rinv{b}")
        nc.vector.reciprocal(rinv[:], rowsum[:])
        rinvs.append(rinv)

        # ---- attn^T tiles and PV accumulation ----
        for jt in range(nj):
            at_ps = tps.tile([128, QPC], f32, tag="atps")
            nc.tensor.transpose(
                at_ps[:], E[:, 128 * jt : 128 * (jt + 1)], ident_sb[0:QPC, 0:QPC]
            )
            at_sb = tpool.tile([128, QPC], f32, tag="atsb")
            nc.scalar.copy(at_sb[:], at_ps[:])
            nc.tensor.matmul(
                pv_ps[:, b * QPC : (b + 1) * QPC],
                lhsT=vals_sb[(b, jt)][:],
                rhs=at_sb[:],
                start=(jt == 0),
                stop=(jt == nj - 1),
            )

    # ---- output projection, with 1/rowsum folded into the PSUM->SBUF copy ----
    pv_sb = consts.tile([D, B * QPC], f32, tag="pvsb")
    nc.scalar.copy(pv_sb[:], pv_ps[:])
    outps = ctx.enter_context(tc.tile_pool(name="outps", bufs=2, space="PSUM"))
    for b in range(B):
        o_ps = outps.tile([QPC, H], f32, tag="ops")
        nc.tensor.matmul(
            o_ps[:],
            lhsT=pv_sb[:, b * QPC : (b + 1) * QPC],
            rhs=wo_sb[:],
            start=True,
            stop=True,
        )
        o_sb = opool.tile([QPC, H], f32, tag="osb")
        nc.scalar.activation(o_sb[:], o_ps[:], AF.Copy, scale=rinvs[b][:])
        nc.sync.dma_start(out_d[b * QPC : (b + 1) * QPC, :], o_sb[:])


def _build_program(valid: tuple, iters: int = 1):
    import concourse.bacc as bacc
    import concourse.mybir as mybir
    import concourse.tile as tile

    f32 = mybir.dt.float32
    AF = mybir.ActivationFunctionType
    AX = mybir.AxisListType

    nc = bacc.Bacc("TRN2", target_bir_lowering=False, debug=False)

    dram = (
        nc.dram_tensor("qT", [D, B * QPC], f32, kind="ExternalInput"),
        nc.dram_tensor("kT", [B * D, LK], f32, kind="ExternalInput"),
        nc.dram_tensor("vals", [B * LK, D], f32, kind="ExternalInput"),
        nc.dram_tensor("wq", [D, H], f32, kind="ExternalInput"),
        nc.dram_tensor("wk", [D, H], f32, kind="ExternalInput"),
        nc.dram_tensor("wo", [D, H], f32, kind="ExternalInput"),
        nc.dram_tensor("zmat", [H, 63], f32, kind="ExternalInput"),
        nc.dram_tensor("ident", [128, 128], f32, kind="ExternalInput"),
        nc.dram_tensor("out", [B * QPC, H], f32, kind="ExternalOutput"),
    )

    njs = [max(1, math.ceil(v / 128)) for v in valid]

    with tile.TileContext(nc) as tc, ExitStack() as ctx:
        consts = ctx.enter_context(tc.tile_pool(name="consts", bufs=1))
        if iters == 1:
            _emit_body(nc, tc, ctx, consts, valid, njs, dram, f32, AF, AX)
        else:
            with tc.For_i(0, iters, 1):
                with ExitStack() as ictx:
                    _emit_body(nc, tc, ictx, consts, valid, njs, dram, f32, AF, AX)

    nc.compile()
    return nc


class Runner:
    """Cached jitted shard_map over the 8 cores, reusable across calls."""

    def __init__(self, nc):
        import jax
        import concourse.mybir as mybir
        from concourse import bass2jax
        from jax.sharding import Mesh, PartitionSpec
        from jax.experimental.shard_map import shard_map

        bass2jax.install_neuronx_cc_hook()
        self.jax = jax

        partition_name = nc.partition_id_tensor.name if nc.partition_id_tensor else None
        in_names, out_names, out_avals, zero_outs = [], [], [], []
        for alloc in nc.m.functions[0].allocations:
            if not isinstance(alloc, mybir.MemoryLocationSet):
                continue
            name = alloc.memorylocations[0].name
            if alloc.kind == "ExternalInput":
                if name != partition_name:
                    in_names.append(name)
            elif alloc.kind == "ExternalOutput":
                out_names.append(name)
                shape = tuple(alloc.tensor_shape)
                dtype = mybir.dt.np(alloc.dtype)
                out_avals.append(jax.core.ShapedArray(shape, dtype))
                zero_outs.append(np.zeros(shape, dtype))
        self.in_names = in_names
        self.n_params = len(in_names)
        n_outs = len(out_avals)
        all_in_names = in_names + out_names
        if partition_name is not None:
            all_in_names = all_in_names + [partition_name]
        self.out_names = out_names
        self.out_avals = out_avals
        self.zero_outs = zero_outs

        def _body(*args):
            operands = list(args)
            if partition_name is not None:
                operands.append(bass2jax.partition_id_tensor())
            outs = bass2jax._bass_exec_p.bind(
                *operands,
                out_avals=tuple(out_avals),
                in_names=tuple(all_in_names),
                out_names=tuple(out_names),
                lowering_input_output_aliases=(),
                sim_require_finite=True,
                sim_require_nnan=True,
                nc=nc,
            )
            return tuple(outs)

        devices = jax.devices()[:NCORES]
        mesh = Mesh(np.asarray(devices), ("core",))
        n_all = self.n_params + n_outs
        self.fn = jax.jit(
            shard_map(
                _body,
                mesh=mesh,
                in_specs=(PartitionSpec("core"),) * n_all,
                out_specs=(PartitionSpec("core"),) * n_outs,
                check_rep=False,
            ),
            donate_argnums=tuple(range(self.n_params, n_all)),
            keep_unused=True,
        )

    def stage_inputs(self, in_maps):
        per_core = [[np.asarray(m[name]) for name in self.in_names] for m in in_maps]
        return [
            self.jax.device_put(
                np.concatenate([per_core[c][i] for c in range(NCORES)], axis=0)
            )
            for i in range(self.n_params)
        ]

    def fresh_zeros(self):
        return [
            self.jax.device_put(np.zeros((NCORES * z.shape[0], *z.shape[1:]), z.dtype))
            for z in self.zero_outs
        ]

    def run(self, staged_inputs):
        outs = self.fn(*staged_inputs, *self.fresh_zeros())
        self.jax.block_until_ready(outs)
        i = self.out_names.index("out")
        return [
            np.asarray(outs[i]).reshape(NCORES, *self.out_avals[i].shape)[c]
            for c in range(NCORES)
        ]


def _get_runner(valid: tuple, iters: int = 1):
    key = (valid, iters)
    if key not in _RUNNERS:
        _RUNNERS[key] = Runner(_build_program(valid, iters))
    return _RUNNERS[key]


def make_in_maps(queries, keys, values, valid_seq_len, Wq, Wk, wv, Wo):
    queries = np.asarray(queries, np.float32)
    keys = np.asarray(keys, np.float32)
    values = np.asarray(values, np.float32)
    Wq = np.ascontiguousarray(np.asarray(Wq, np.float32))
    Wk = np.ascontiguousarray(np.asarray(Wk, np.float32))
    wv = np.asarray(wv, np.float32)
    Wo = np.ascontiguousarray(np.asarray(Wo, np.float32))

    qT_full = np.ascontiguousarray(queries.transpose(2, 0, 1))  # (D, B, Lq)
    kT = np.ascontiguousarray(keys.transpose(0, 2, 1)).reshape(B * D, LK)
    vals = np.ascontiguousarray(values.reshape(B * LK, D))
    zmat = np.zeros((H, 63), np.float32)
    zmat[:, 31] = wv
    ident = np.eye(128, dtype=np.float32)

    in_maps = []
    for c in range(NCORES):
        qT_c = np.ascontiguousarray(
            qT_full[:, :, c * QPC : (c + 1) * QPC].reshape(D, B * QPC)
        )
        in_maps.append(
            dict(qT=qT_c, kT=kT, vals=vals, wq=Wq, wk=Wk, wo=Wo, zmat=zmat, ident=ident)
        )
    return in_maps


def assemble(outs):
    out = np.empty((B, LQ, H), np.float32)
    for c in range(NCORES):
        out[:, c * QPC : (c + 1) * QPC, :] = outs[c].reshape(B, QPC, H)
    return out


def kernel(queries, keys, values, valid_seq_len, Wq, Wk, wv, Wo):
    valid = tuple(int(v) for v in np.asarray(valid_seq_len))
    runner = _get_runner(valid)
    in_maps = make_in_maps(queries, keys, values, valid_seq_len, Wq, Wk, wv, Wo)
    return assemble(runner.run(runner.stage_inputs(in_maps)))
